# revision 47
# baseline (speedup 1.0000x reference)
"""Differential Multi-Head Attention on 8 Trainium2 NeuronCores.

Sharding: core c = 4*b + hg handles batch b (2) x head-group hg (4 heads each).
Each core computes, for its 4 heads: QKV projections, two attention maps,
diff_attn = softmax(Q1K1')-lam*softmax(Q2K2') (written f32 to HBM), and the
partial output contribution attn @ V -> RMSNorm -> @ Wproj rows (host sums the
4 head-group partials per batch).

Layout choices:
  - x is fed pre-transposed (xT [C, T]) so Q^T/K^T project directly with W as
    the stationary operand.
  - Q/K for the two maps are interleaved on partitions (map1 at 0:64, map2 at
    64:128) via host-packed Wqmix/Wkmix, enabling row-packed concurrent score
    matmuls.
  - exp runs on ACT with accum_out giving softmax row-sums for free;
    normalization+subtraction fuse into tensor_scalar (gpsimd) +
    scalar_tensor_tensor (DVE).
  - attn @ V needs k on partitions: diff tiles are cast to bf16 (gpsimd) and
    transposed SBUF->SBUF by the DMA xbar, then contracted against bf16 V.
  - RMSNorm runs in [d, q] layout: per-head sums of squares via a 0/1-mask
    matmul, rsqrt via ln/exp (same ACT table set), broadcast back over
    partitions via a K=2 matmul against the mask transpose.
"""

import sys

if "/opt/trn_rl_repo" not in sys.path:
    sys.path.insert(0, "/opt/trn_rl_repo")

from contextlib import ExitStack

import numpy as np

import concourse.bass as bass
import concourse.mybir as mybir
import concourse.tile as tile
from concourse.bass_utils import run_bass_kernel_spmd

F32 = mybir.dt.float32
F32R = mybir.dt.float32r
BF16 = mybir.dt.bfloat16
AF = mybir.ActivationFunctionType
ALU = mybir.AluOpType

B, T, C, H = 2, 2048, 1024, 16
D = C // H  # 64
HG = 4  # head-groups (cores per batch)
HPG = H // HG  # heads per group = 4
DG = HPG * D  # 256 d-cols per group
LAMBDA_INIT = 0.8 - 0.6 * float(np.exp(-0.3 * 0.0))  # 0.2
EPS = 1e-8
SCALE = float(D) ** -0.5

NQC = T // 128  # 16 q-chunks
NKC = T // 128  # 16 k-chunks
NKH = 2  # k-halves of 1024
GQC = 2  # q-chunks per transpose/U group
NG = NQC // GQC  # 4 groups

_CACHE = {}


def _split_sem_waits(nc, max_waits=1):
    """neuronxcc walrus in this container rejects >1 sync wait per
    instruction; move excess waits onto preceding same-engine NoOps
    (same-engine waits run in program order, so semantics are unchanged)."""
    for bb in nc.main_func.blocks:
        new_list = []
        for ins in bb.instructions:
            si = ins.sync_info
            if si is not None and si.on_wait and len(si.on_wait) > max_waits:
                waits = list(si.on_wait)
                extra, keep = waits[:-max_waits], waits[-max_waits:]
                for i in range(0, len(extra), max_waits):
                    nop = mybir.InstNoOp(name=f"{ins.name}_wsplit_{i}")
                    nop.engine = ins.engine
                    nop.sync_info = mybir.SyncInfo(
                        on_wait=extra[i : i + max_waits], on_update=[]
                    )
                    new_list.append(nop)
                si.on_wait = keep
            new_list.append(ins)
        bb.instructions[:] = new_list


def _build_program():
    nc = bass.Bass()

    xt_d = nc.dram_tensor("xt", [C, T], F32, kind="ExternalInput")
    wqm_d = nc.dram_tensor("wqm", [C, 2 * DG], F32, kind="ExternalInput")
    wkm_d = nc.dram_tensor("wkm", [C, 2 * DG], F32, kind="ExternalInput")
    wv_d = nc.dram_tensor("wv", [C, DG], F32, kind="ExternalInput")
    wpj_d = nc.dram_tensor("wpj", [DG, C], F32, kind="ExternalInput")
    lam_d = nc.dram_tensor("lam", [128, 1], F32, kind="ExternalInput")
    hmask_d = nc.dram_tensor("hmask", [128, 2], F32, kind="ExternalInput")
    hmaskT_d = nc.dram_tensor("hmaskT", [2, 128], F32, kind="ExternalInput")

    diff_d = nc.dram_tensor("diff", [HPG, T, T], F32, kind="ExternalOutput")
    outp_d = nc.dram_tensor("outp", [T, C], F32, kind="ExternalOutput")

    xt_v = xt_d.rearrange("(cc p) t -> p cc t", p=128)  # [128, 8, T]
    wqm_v = wqm_d.rearrange("(cc p) m -> p cc m", p=128)  # [128, 8, 512]
    wkm_v = wkm_d.rearrange("(cc p) m -> p cc m", p=128)
    wv_v = wv_d.rearrange("(cc p) m -> p cc m", p=128)  # [128, 8, 256]
    wpj_v = wpj_d.rearrange("(hp p) n -> p hp n", p=128)  # [128, 2, 1024]

    with tile.TileContext(nc) as tc, ExitStack() as es:
        consts = es.enter_context(tc.tile_pool(name="consts", bufs=1))

        lam_s = consts.tile([128, 1], F32)
        hmask_s = consts.tile([128, 2], F32)
        hmaskT_s = consts.tile([2, 128], F32)
        nc.sync.dma_start(out=lam_s, in_=lam_d[:])
        nc.sync.dma_start(out=hmask_s, in_=hmask_d[:])
        nc.sync.dma_start(out=hmaskT_s, in_=hmaskT_d[:])

        qmix = [
            consts.tile([128, T], F32R, name=f"qmix{j}", tag=f"qmix{j}")
            for j in range(HPG)
        ]
        kmix = [
            consts.tile([128, T], F32R, name=f"kmix{j}", tag=f"kmix{j}")
            for j in range(HPG)
        ]
        v_bf = consts.tile([128, NKC, DG], BF16)
        uT_sb = consts.tile([128, 2, T], F32R)  # [d-pair, hp, q]
        rms2 = consts.tile([2, T], F32)
        eps_t = consts.tile([2, 1], F32)
        nc.vector.memset(eps_t, EPS)

        # ---------------- P1: projections ----------------
        with tc.tile_pool(name="p1w", bufs=1) as p1w, tc.tile_pool(
            name="p1x", bufs=2
        ) as p1x, tc.tile_pool(name="p1ps", bufs=2, space="PSUM") as p1ps:
            wqm_s = p1w.tile([128, 8, 2 * DG], F32R)
            wkm_s = p1w.tile([128, 8, 2 * DG], F32R)
            wv_s = p1w.tile([128, 8, DG], F32R)
            nc.gpsimd.dma_start(out=wqm_s, in_=wqm_v)
            nc.gpsimd.dma_start(out=wkm_s, in_=wkm_v)
            nc.gpsimd.dma_start(out=wv_s, in_=wv_v)
            for tb in range(4):  # t-blocks of 512
                ts0 = tb * 512
                xt_t = p1x.tile([128, 8, 512], F32R)
                nc.gpsimd.dma_start(out=xt_t, in_=xt_v[:, :, ts0 : ts0 + 512])
                for j in range(HPG):
                    psq = p1ps.tile([128, 512], F32, tag="psq")
                    for cc in range(8):
                        nc.tensor.matmul(
                            psq,
                            wqm_s[:, cc, 128 * j : 128 * j + 128],
                            xt_t[:, cc, :],
                            start=(cc == 0),
                            stop=(cc == 7),
                        )
                    nc.vector.tensor_copy(out=qmix[j][:, ts0 : ts0 + 512], in_=psq)
                    psk = p1ps.tile([128, 512], F32, tag="psk")
                    for cc in range(8):
                        nc.tensor.matmul(
                            psk,
                            wkm_s[:, cc, 128 * j : 128 * j + 128],
                            xt_t[:, cc, :],
                            start=(cc == 0),
                            stop=(cc == 7),
                        )
                    nc.vector.tensor_copy(out=kmix[j][:, ts0 : ts0 + 512], in_=psk)
                for t4 in range(4):  # t-128-chunks inside tb
                    psv = p1ps.tile([128, DG], F32, tag="psv")
                    for cc in range(8):
                        nc.tensor.matmul(
                            psv,
                            xt_t[:, cc, 128 * t4 : 128 * t4 + 128],
                            wv_s[:, cc, :],
                            start=(cc == 0),
                            stop=(cc == 7),
                        )
                    nc.vector.tensor_copy(out=v_bf[:, 4 * tb + t4, :], in_=psv)

        # ---------------- P2/P3: attention ----------------
        with tc.tile_pool(name="att_e", bufs=3) as att_e, tc.tile_pool(
            name="att_sm", bufs=3
        ) as att_sm, tc.tile_pool(name="att_dh", bufs=2) as att_dh, tc.tile_pool(
            name="att_dbf", bufs=2
        ) as att_dbf, tc.tile_pool(name="att_dT", bufs=1) as att_dT, tc.tile_pool(
            name="att_ps", bufs=3, space="PSUM"
        ) as att_ps, tc.tile_pool(name="u_ps", bufs=2, space="PSUM") as u_ps:
            def emit_qc(j, qc, diffT_g, qi):
                q0 = qc * 128
                e1 = att_e.tile([128, T], F32, tag="e1", bufs=3, name=f"e1_{j}_{qc}")
                e2 = att_e.tile([128, T], F32, tag="e2", bufs=3, name=f"e2_{j}_{qc}")
                accs = att_sm.tile([128, 2, 2], F32, tag="accs", name=f"ac_{j}_{qc}")
                for m in range(2):
                    lhs = qmix[j][64 * m : 64 * m + 64, q0 : q0 + 128]
                    for kh in range(NKH):
                        k0 = kh * 1024
                        sp = att_ps.tile(
                            [128, 1024], F32, tag="sp", name=f"sp_{j}_{qc}_{m}_{kh}"
                        )
                        for ks in range(2):
                            nc.tensor.matmul(
                                sp[:, 512 * ks : 512 * ks + 512],
                                lhs,
                                kmix[j][
                                    64 * m : 64 * m + 64,
                                    k0 + 512 * ks : k0 + 512 * ks + 512,
                                ],
                                start=True,
                                stop=True,
                                tile_position=(64 * m, 0),
                            )
                        nc.scalar.activation(
                            out=(e1 if m == 0 else e2)[:, k0 : k0 + 1024],
                            in_=sp,
                            func=AF.Exp,
                            scale=SCALE,
                            accum_out=accs[:, m, kh : kh + 1],
                        )
                sums = att_sm.tile([128, 2], F32, tag="sums", name=f"su_{j}_{qc}")
                nc.vector.tensor_tensor(
                    out=sums, in0=accs[:, :, 0], in1=accs[:, :, 1], op=ALU.add
                )
                rcp = att_sm.tile([128, 2], F32, tag="rcp", name=f"rc_{j}_{qc}")
                nc.vector.reciprocal(out=rcp, in_=sums)
                dbf = att_dbf.tile([128, T], BF16, tag="dbf", bufs=2, name=f"db_{j}_{qc}")
                dh = att_dh.tile([128, T], F32, tag="dh", bufs=3, name=f"dh_{j}_{qc}")
                for kh in range(NKH):
                    k0 = kh * 1024
                    nc.vector.tensor_scalar(
                        out=e2[:, k0 : k0 + 1024],
                        in0=e2[:, k0 : k0 + 1024],
                        scalar1=rcp[:, 1:2],
                        scalar2=lam_s,
                        op0=ALU.mult,
                        op1=ALU.mult,
                    )
                    nc.vector.scalar_tensor_tensor(
                        out=dh[:, k0 : k0 + 1024],
                        in0=e1[:, k0 : k0 + 1024],
                        scalar=rcp[:, 0:1],
                        in1=e2[:, k0 : k0 + 1024],
                        op0=ALU.mult,
                        op1=ALU.subtract,
                    )
                nc.sync.dma_start(out=diff_d[j, q0 : q0 + 128, :], in_=dh)
                nc.gpsimd.tensor_copy(out=dbf, in_=dh)
                nc.sync.dma_start_transpose(
                    out=diffT_g[:, :, 128 * qi : 128 * qi + 128], in_=dbf
                )

            def emit_u(j, g, diffT_g):
                pj = j % 2
                hp = j // 2
                up = u_ps.tile([128, 128 * GQC], F32, tag="up", name=f"up_{j}_{g}")
                for kc in range(NKC):
                    nc.tensor.matmul(
                        up[64 * pj : 64 * pj + 64, :],
                        v_bf[:, kc, 64 * j : 64 * j + 64],
                        diffT_g[:, kc, :],
                        start=(kc == 0),
                        stop=(kc == NKC - 1),
                        tile_position=(0, 64 * pj),
                    )
                q0 = 128 * GQC * g
                nc.vector.tensor_copy(
                    out=uT_sb[64 * pj : 64 * pj + 64, hp, q0 : q0 + 128 * GQC],
                    in_=up[64 * pj : 64 * pj + 64, :],
                )

            def emit_rms_front(hpi):
                # squares (into a borrowed dh-ring slot) + per-head
                # sum-of-squares + in-place rsqrt + in-place apply
                sq_t = att_dh.tile([128, T], F32, tag="dh", bufs=3, name=f"sq{hpi}")
                nc.scalar.activation(
                    out=sq_t, in_=uT_sb[:, hpi, :], func=AF.Square, scale=1.0
                )
                for qb in range(4):
                    ssq_ps = u_ps.tile(
                        [128, 512], F32, tag="up", name=f"sqp_{hpi}_{qb}"
                    )
                    nc.tensor.matmul(
                        ssq_ps[0:2, :],
                        hmask_s,
                        sq_t[:, 512 * qb : 512 * qb + 512],
                        start=True,
                        stop=True,
                    )
                    nc.vector.tensor_copy(
                        out=rms2[:, 512 * qb : 512 * qb + 512],
                        in_=ssq_ps[0:2, :],
                    )
                nc.scalar.activation(
                    out=rms2, in_=rms2, func=AF.Ln, scale=1.0 / D, bias=eps_t
                )
                nc.scalar.activation(out=rms2, in_=rms2, func=AF.Exp, scale=-0.5)
                for qb in range(4):
                    bc_ps = u_ps.tile(
                        [128, 512], F32, tag="up", name=f"bcp_{hpi}_{qb}"
                    )
                    nc.tensor.matmul(
                        bc_ps,
                        hmaskT_s,
                        rms2[:, 512 * qb : 512 * qb + 512],
                        start=True,
                        stop=True,
                    )
                    nc.vector.tensor_tensor(
                        out=uT_sb[:, hpi, 512 * qb : 512 * qb + 512],
                        in0=uT_sb[:, hpi, 512 * qb : 512 * qb + 512],
                        in1=bc_ps,
                        op=ALU.mult,
                    )

            for jp in range(2):  # head pairs, two interleaved streams each
                for g in range(NG):
                    dts = [
                        att_dT.tile(
                            [128, NKC, 128 * GQC],
                            BF16,
                            tag="dTg",
                            bufs=3,
                            name=f"dT_{jp}_{g}_{j2}",
                        )
                        for j2 in range(2)
                    ]
                    for qi in range(GQC):
                        for j2 in range(2):
                            emit_qc(2 * jp + j2, g * GQC + qi, dts[j2], qi)
                    for j2 in range(2):
                        emit_u(2 * jp + j2, g, dts[j2])
                emit_rms_front(jp)

        # ---------------- P4: RMSNorm + projection ----------------
        with tc.tile_pool(name="p4", bufs=1) as p4, tc.tile_pool(
            name="p4o", bufs=2
        ) as p4o, tc.tile_pool(name="p4ps", bufs=2, space="PSUM") as p4ps:
            wpj_s = p4.tile([128, 2, C], F32R)
            nc.gpsimd.dma_start(out=wpj_s, in_=wpj_v)
            # projection: outp[q, n] = sum_d attn_n^T[d, q] * Wproj'[d, n]
            for qc in range(NQC):
                q0 = qc * 128
                ob = p4o.tile([128, C], F32, tag="ob")
                for nh in range(2):
                    pp = p4ps.tile([128, 512], F32, tag="pp")
                    for hpi in range(2):
                        nc.tensor.matmul(
                            pp,
                            uT_sb[:, hpi, q0 : q0 + 128],
                            wpj_s[:, hpi, 512 * nh : 512 * nh + 512],
                            start=(hpi == 0),
                            stop=(hpi == 1),
                        )
                    nc.vector.tensor_copy(out=ob[:, 512 * nh : 512 * nh + 512], in_=pp)
                nc.sync.dma_start(out=outp_d[q0 : q0 + 128, :], in_=ob)

    _split_sem_waits(nc)
    return nc


def _get_program():
    if "nc" not in _CACHE:
        _CACHE["nc"] = _build_program()
    return _CACHE["nc"]


def kernel(x, Wq1, Wq2, Wk1, Wk2, Wv, Wproj, lq1, lk1, lq2, lk2, norm_w):
    x = np.asarray(x, np.float32)
    Wq1, Wq2 = np.asarray(Wq1, np.float32), np.asarray(Wq2, np.float32)
    Wk1, Wk2 = np.asarray(Wk1, np.float32), np.asarray(Wk2, np.float32)
    Wv, Wproj = np.asarray(Wv, np.float32), np.asarray(Wproj, np.float32)
    lq1, lk1 = np.asarray(lq1, np.float32), np.asarray(lk1, np.float32)
    lq2, lk2 = np.asarray(lq2, np.float32), np.asarray(lk2, np.float32)
    norm_w = np.asarray(norm_w, np.float32)

    lam = float(
        np.exp(np.sum(lq1 * lk1, dtype=np.float64))
        - np.exp(np.sum(lq2 * lk2, dtype=np.float64))
        + LAMBDA_INIT
    )

    # norm_w * (1 - LAMBDA_INIT) folded into Wproj rows
    normw_t = np.tile(norm_w, HPG) * (1.0 - LAMBDA_INIT)  # [DG]

    hmask = np.zeros((128, 2), np.float32)
    hmask[0:64, 0] = 1.0
    hmask[64:128, 1] = 1.0
    hmaskT = np.ascontiguousarray(hmask.T)
    lam_v = np.full((128, 1), lam, np.float32)

    in_maps = []
    for c in range(8):
        b, hg = divmod(c, HG)
        xt = np.ascontiguousarray(x[b].T)  # [C, T]
        wqm = np.empty((C, 2 * DG), np.float32)
        wkm = np.empty((C, 2 * DG), np.float32)
        for jj in range(HPG):
            gh = HPG * hg + jj
            wqm[:, 128 * jj : 128 * jj + 64] = Wq1[:, 64 * gh : 64 * gh + 64]
            wqm[:, 128 * jj + 64 : 128 * jj + 128] = Wq2[:, 64 * gh : 64 * gh + 64]
            wkm[:, 128 * jj : 128 * jj + 64] = Wk1[:, 64 * gh : 64 * gh + 64]
            wkm[:, 128 * jj + 64 : 128 * jj + 128] = Wk2[:, 64 * gh : 64 * gh + 64]
        wv = np.ascontiguousarray(Wv[:, DG * hg : DG * hg + DG])
        wpj = np.ascontiguousarray(
            normw_t[:, None] * Wproj[DG * hg : DG * hg + DG, :]
        )
        in_maps.append(
            {
                "xt": xt,
                "wqm": wqm,
                "wkm": wkm,
                "wv": wv,
                "wpj": wpj,
                "lam": lam_v,
                "hmask": hmask,
                "hmaskT": hmaskT,
            }
        )

    nc = _get_program()
    res = run_bass_kernel_spmd(nc, in_maps, list(range(8))).results

    diff_attn = np.empty((B, H, T, T), np.float32)
    out = np.zeros((B, T, C), np.float32)
    for c in range(8):
        b, hg = divmod(c, HG)
        diff_attn[b, HPG * hg : HPG * hg + HPG] = res[c]["diff"]
        out[b] += res[c]["outp"]
    return out, diff_attn


# revision 48
# speedup vs baseline: 1.0006x; 1.0006x over previous
"""Differential Multi-Head Attention on 8 Trainium2 NeuronCores.

Sharding: core c = 4*b + hg handles batch b (2) x head-group hg (4 heads each).
Each core computes, for its 4 heads: QKV projections, two attention maps,
diff_attn = softmax(Q1K1')-lam*softmax(Q2K2') (written f32 to HBM), and the
partial output contribution attn @ V -> RMSNorm -> @ Wproj rows (host sums the
4 head-group partials per batch).

Layout choices:
  - x is fed pre-transposed (xT [C, T]) so Q^T/K^T project directly with W as
    the stationary operand.
  - Q/K for the two maps are interleaved on partitions (map1 at 0:64, map2 at
    64:128) via host-packed Wqmix/Wkmix, enabling row-packed concurrent score
    matmuls.
  - exp runs on ACT with accum_out giving softmax row-sums for free;
    normalization+subtraction fuse into tensor_scalar (gpsimd) +
    scalar_tensor_tensor (DVE).
  - attn @ V needs k on partitions: diff tiles are cast to bf16 (gpsimd) and
    transposed SBUF->SBUF by the DMA xbar, then contracted against bf16 V.
  - RMSNorm runs in [d, q] layout: per-head sums of squares via a 0/1-mask
    matmul, rsqrt via ln/exp (same ACT table set), broadcast back over
    partitions via a K=2 matmul against the mask transpose.
"""

import sys

if "/opt/trn_rl_repo" not in sys.path:
    sys.path.insert(0, "/opt/trn_rl_repo")

from contextlib import ExitStack

import numpy as np

import concourse.bass as bass
import concourse.mybir as mybir
import concourse.tile as tile
from concourse.bass_utils import run_bass_kernel_spmd

F32 = mybir.dt.float32
F32R = mybir.dt.float32r
BF16 = mybir.dt.bfloat16
AF = mybir.ActivationFunctionType
ALU = mybir.AluOpType

B, T, C, H = 2, 2048, 1024, 16
D = C // H  # 64
HG = 4  # head-groups (cores per batch)
HPG = H // HG  # heads per group = 4
DG = HPG * D  # 256 d-cols per group
LAMBDA_INIT = 0.8 - 0.6 * float(np.exp(-0.3 * 0.0))  # 0.2
EPS = 1e-8
SCALE = float(D) ** -0.5

NQC = T // 128  # 16 q-chunks
NKC = T // 128  # 16 k-chunks
NKH = 2  # k-halves of 1024
GQC = 2  # q-chunks per transpose/U group
NG = NQC // GQC  # 4 groups

_CACHE = {}


def _split_sem_waits(nc, max_waits=1):
    """neuronxcc walrus in this container rejects >1 sync wait per
    instruction; move excess waits onto preceding same-engine NoOps
    (same-engine waits run in program order, so semantics are unchanged)."""
    for bb in nc.main_func.blocks:
        new_list = []
        for ins in bb.instructions:
            si = ins.sync_info
            if si is not None and si.on_wait and len(si.on_wait) > max_waits:
                waits = list(si.on_wait)
                extra, keep = waits[:-max_waits], waits[-max_waits:]
                for i in range(0, len(extra), max_waits):
                    nop = mybir.InstNoOp(name=f"{ins.name}_wsplit_{i}")
                    nop.engine = ins.engine
                    nop.sync_info = mybir.SyncInfo(
                        on_wait=extra[i : i + max_waits], on_update=[]
                    )
                    new_list.append(nop)
                si.on_wait = keep
            new_list.append(ins)
        bb.instructions[:] = new_list


def _build_program():
    nc = bass.Bass()

    xt_d = nc.dram_tensor("xt", [C, T], F32, kind="ExternalInput")
    wqm_d = nc.dram_tensor("wqm", [C, 2 * DG], F32, kind="ExternalInput")
    wkm_d = nc.dram_tensor("wkm", [C, 2 * DG], F32, kind="ExternalInput")
    wv_d = nc.dram_tensor("wv", [C, DG], F32, kind="ExternalInput")
    wpj_d = nc.dram_tensor("wpj", [DG, C], F32, kind="ExternalInput")
    lam_d = nc.dram_tensor("lam", [128, 1], F32, kind="ExternalInput")
    hmask_d = nc.dram_tensor("hmask", [128, 2], F32, kind="ExternalInput")
    hmaskT_d = nc.dram_tensor("hmaskT", [2, 128], F32, kind="ExternalInput")

    diff_d = nc.dram_tensor("diff", [HPG, T, T], F32, kind="ExternalOutput")
    outp_d = nc.dram_tensor("outp", [T, C], F32, kind="ExternalOutput")

    xt_v = xt_d.rearrange("(cc p) t -> p cc t", p=128)  # [128, 8, T]
    wqm_v = wqm_d.rearrange("(cc p) m -> p cc m", p=128)  # [128, 8, 512]
    wkm_v = wkm_d.rearrange("(cc p) m -> p cc m", p=128)
    wv_v = wv_d.rearrange("(cc p) m -> p cc m", p=128)  # [128, 8, 256]
    wpj_v = wpj_d.rearrange("(hp p) n -> p hp n", p=128)  # [128, 2, 1024]

    with tile.TileContext(nc) as tc, ExitStack() as es:
        consts = es.enter_context(tc.tile_pool(name="consts", bufs=1))

        lam_s = consts.tile([128, 1], F32)
        hmask_s = consts.tile([128, 2], F32)
        hmaskT_s = consts.tile([2, 128], F32)
        nc.sync.dma_start(out=lam_s, in_=lam_d[:])
        nc.sync.dma_start(out=hmask_s, in_=hmask_d[:])
        nc.sync.dma_start(out=hmaskT_s, in_=hmaskT_d[:])

        qmix = [
            consts.tile([128, T], F32R, name=f"qmix{j}", tag=f"qmix{j}")
            for j in range(HPG)
        ]
        kmix = [
            consts.tile([128, T], F32R, name=f"kmix{j}", tag=f"kmix{j}")
            for j in range(HPG)
        ]
        v_bf = consts.tile([128, NKC, DG], BF16)
        uT_sb = consts.tile([128, 2, T], F32R)  # [d-pair, hp, q]
        rms2 = consts.tile([2, T], F32)
        eps_t = consts.tile([2, 1], F32)
        nc.vector.memset(eps_t, EPS)

        # ---------------- P1: projections ----------------
        with tc.tile_pool(name="p1w", bufs=1) as p1w, tc.tile_pool(
            name="p1x", bufs=2
        ) as p1x, tc.tile_pool(name="p1ps", bufs=2, space="PSUM") as p1ps:
            wqm_s = p1w.tile([128, 8, 2 * DG], F32R)
            wkm_s = p1w.tile([128, 8, 2 * DG], F32R)
            wv_s = p1w.tile([128, 8, DG], F32R)
            nc.gpsimd.dma_start(out=wqm_s, in_=wqm_v)
            nc.gpsimd.dma_start(out=wkm_s, in_=wkm_v)
            nc.gpsimd.dma_start(out=wv_s, in_=wv_v)
            for tb in range(4):  # t-blocks of 512
                ts0 = tb * 512
                xt_t = p1x.tile([128, 8, 512], F32R)
                nc.gpsimd.dma_start(out=xt_t, in_=xt_v[:, :, ts0 : ts0 + 512])
                for j in range(HPG):
                    psq = p1ps.tile([128, 512], F32, tag="psq")
                    for cc in range(8):
                        nc.tensor.matmul(
                            psq,
                            wqm_s[:, cc, 128 * j : 128 * j + 128],
                            xt_t[:, cc, :],
                            start=(cc == 0),
                            stop=(cc == 7),
                        )
                    nc.vector.tensor_copy(out=qmix[j][:, ts0 : ts0 + 512], in_=psq)
                    psk = p1ps.tile([128, 512], F32, tag="psk")
                    for cc in range(8):
                        nc.tensor.matmul(
                            psk,
                            wkm_s[:, cc, 128 * j : 128 * j + 128],
                            xt_t[:, cc, :],
                            start=(cc == 0),
                            stop=(cc == 7),
                        )
                    nc.vector.tensor_copy(out=kmix[j][:, ts0 : ts0 + 512], in_=psk)
                for t4 in range(4):  # t-128-chunks inside tb
                    psv = p1ps.tile([128, DG], F32, tag="psv")
                    for cc in range(8):
                        nc.tensor.matmul(
                            psv,
                            xt_t[:, cc, 128 * t4 : 128 * t4 + 128],
                            wv_s[:, cc, :],
                            start=(cc == 0),
                            stop=(cc == 7),
                        )
                    nc.vector.tensor_copy(out=v_bf[:, 4 * tb + t4, :], in_=psv)

        # ---------------- P2/P3: attention ----------------
        with tc.tile_pool(name="att_e", bufs=3) as att_e, tc.tile_pool(
            name="att_sm", bufs=3
        ) as att_sm, tc.tile_pool(name="att_dh", bufs=2) as att_dh, tc.tile_pool(
            name="att_dbf", bufs=2
        ) as att_dbf, tc.tile_pool(name="att_dT", bufs=1) as att_dT, tc.tile_pool(
            name="att_ps", bufs=3, space="PSUM"
        ) as att_ps, tc.tile_pool(name="u_ps", bufs=2, space="PSUM") as u_ps:
            def emit_qc(j, qc, diffT_g, qi):
                q0 = qc * 128
                e1 = att_e.tile([128, T], F32, tag="e1", bufs=3, name=f"e1_{j}_{qc}")
                e2 = att_e.tile([128, T], F32, tag="e2", bufs=3, name=f"e2_{j}_{qc}")
                accs = att_sm.tile([128, 2, 2], F32, tag="accs", name=f"ac_{j}_{qc}")
                for m in range(2):
                    lhs = qmix[j][64 * m : 64 * m + 64, q0 : q0 + 128]
                    for kh in range(NKH):
                        k0 = kh * 1024
                        sp = att_ps.tile(
                            [128, 1024], F32, tag="sp", name=f"sp_{j}_{qc}_{m}_{kh}"
                        )
                        for ks in range(2):
                            nc.tensor.matmul(
                                sp[:, 512 * ks : 512 * ks + 512],
                                lhs,
                                kmix[j][
                                    64 * m : 64 * m + 64,
                                    k0 + 512 * ks : k0 + 512 * ks + 512,
                                ],
                                start=True,
                                stop=True,
                                tile_position=(64 * m, 0),
                            )
                        nc.scalar.activation(
                            out=(e1 if m == 0 else e2)[:, k0 : k0 + 1024],
                            in_=sp,
                            func=AF.Exp,
                            scale=SCALE,
                            accum_out=accs[:, m, kh : kh + 1],
                        )
                sums = att_sm.tile([128, 2], F32, tag="sums", name=f"su_{j}_{qc}")
                nc.vector.tensor_tensor(
                    out=sums, in0=accs[:, :, 0], in1=accs[:, :, 1], op=ALU.add
                )
                rcp = att_sm.tile([128, 2], F32, tag="rcp", name=f"rc_{j}_{qc}")
                nc.vector.reciprocal(out=rcp, in_=sums)
                dbf = att_dbf.tile([128, T], BF16, tag="dbf", bufs=2, name=f"db_{j}_{qc}")
                dh = att_dh.tile([128, T], F32, tag="dh", bufs=3, name=f"dh_{j}_{qc}")
                for kh in range(NKH):
                    k0 = kh * 1024
                    nc.vector.tensor_scalar(
                        out=e2[:, k0 : k0 + 1024],
                        in0=e2[:, k0 : k0 + 1024],
                        scalar1=rcp[:, 1:2],
                        scalar2=lam_s,
                        op0=ALU.mult,
                        op1=ALU.mult,
                    )
                    nc.vector.scalar_tensor_tensor(
                        out=dh[:, k0 : k0 + 1024],
                        in0=e1[:, k0 : k0 + 1024],
                        scalar=rcp[:, 0:1],
                        in1=e2[:, k0 : k0 + 1024],
                        op0=ALU.mult,
                        op1=ALU.subtract,
                    )
                nc.sync.dma_start(out=diff_d[j, q0 : q0 + 128, :], in_=dh)
                nc.vector.tensor_copy(out=dbf, in_=dh)
                nc.sync.dma_start_transpose(
                    out=diffT_g[:, :, 128 * qi : 128 * qi + 128], in_=dbf
                )

            def emit_u(j, g, diffT_g):
                pj = j % 2
                hp = j // 2
                up = u_ps.tile([128, 128 * GQC], F32, tag="up", name=f"up_{j}_{g}")
                for kc in range(NKC):
                    nc.tensor.matmul(
                        up[64 * pj : 64 * pj + 64, :],
                        v_bf[:, kc, 64 * j : 64 * j + 64],
                        diffT_g[:, kc, :],
                        start=(kc == 0),
                        stop=(kc == NKC - 1),
                        tile_position=(0, 64 * pj),
                    )
                q0 = 128 * GQC * g
                nc.vector.tensor_copy(
                    out=uT_sb[64 * pj : 64 * pj + 64, hp, q0 : q0 + 128 * GQC],
                    in_=up[64 * pj : 64 * pj + 64, :],
                )

            def emit_rms_front(hpi):
                # squares (into a borrowed dh-ring slot) + per-head
                # sum-of-squares + in-place rsqrt + in-place apply
                sq_t = att_dh.tile([128, T], F32, tag="dh", bufs=3, name=f"sq{hpi}")
                nc.scalar.activation(
                    out=sq_t, in_=uT_sb[:, hpi, :], func=AF.Square, scale=1.0
                )
                for qb in range(4):
                    ssq_ps = u_ps.tile(
                        [128, 512], F32, tag="up", name=f"sqp_{hpi}_{qb}"
                    )
                    nc.tensor.matmul(
                        ssq_ps[0:2, :],
                        hmask_s,
                        sq_t[:, 512 * qb : 512 * qb + 512],
                        start=True,
                        stop=True,
                    )
                    nc.vector.tensor_copy(
                        out=rms2[:, 512 * qb : 512 * qb + 512],
                        in_=ssq_ps[0:2, :],
                    )
                nc.scalar.activation(
                    out=rms2, in_=rms2, func=AF.Ln, scale=1.0 / D, bias=eps_t
                )
                nc.scalar.activation(out=rms2, in_=rms2, func=AF.Exp, scale=-0.5)
                for qb in range(4):
                    bc_ps = u_ps.tile(
                        [128, 512], F32, tag="up", name=f"bcp_{hpi}_{qb}"
                    )
                    nc.tensor.matmul(
                        bc_ps,
                        hmaskT_s,
                        rms2[:, 512 * qb : 512 * qb + 512],
                        start=True,
                        stop=True,
                    )
                    nc.vector.tensor_tensor(
                        out=uT_sb[:, hpi, 512 * qb : 512 * qb + 512],
                        in0=uT_sb[:, hpi, 512 * qb : 512 * qb + 512],
                        in1=bc_ps,
                        op=ALU.mult,
                    )

            for jp in range(2):  # head pairs, two interleaved streams each
                for g in range(NG):
                    dts = [
                        att_dT.tile(
                            [128, NKC, 128 * GQC],
                            BF16,
                            tag="dTg",
                            bufs=3,
                            name=f"dT_{jp}_{g}_{j2}",
                        )
                        for j2 in range(2)
                    ]
                    for qi in range(GQC):
                        for j2 in range(2):
                            emit_qc(2 * jp + j2, g * GQC + qi, dts[j2], qi)
                    for j2 in range(2):
                        emit_u(2 * jp + j2, g, dts[j2])
                emit_rms_front(jp)

        # ---------------- P4: RMSNorm + projection ----------------
        with tc.tile_pool(name="p4", bufs=1) as p4, tc.tile_pool(
            name="p4o", bufs=2
        ) as p4o, tc.tile_pool(name="p4ps", bufs=2, space="PSUM") as p4ps:
            wpj_s = p4.tile([128, 2, C], F32R)
            nc.gpsimd.dma_start(out=wpj_s, in_=wpj_v)
            # projection: outp[q, n] = sum_d attn_n^T[d, q] * Wproj'[d, n]
            for qc in range(NQC):
                q0 = qc * 128
                ob = p4o.tile([128, C], F32, tag="ob")
                for nh in range(2):
                    pp = p4ps.tile([128, 512], F32, tag="pp")
                    for hpi in range(2):
                        nc.tensor.matmul(
                            pp,
                            uT_sb[:, hpi, q0 : q0 + 128],
                            wpj_s[:, hpi, 512 * nh : 512 * nh + 512],
                            start=(hpi == 0),
                            stop=(hpi == 1),
                        )
                    nc.vector.tensor_copy(out=ob[:, 512 * nh : 512 * nh + 512], in_=pp)
                nc.sync.dma_start(out=outp_d[q0 : q0 + 128, :], in_=ob)

    _split_sem_waits(nc)
    return nc


def _get_program():
    if "nc" not in _CACHE:
        _CACHE["nc"] = _build_program()
    return _CACHE["nc"]


def kernel(x, Wq1, Wq2, Wk1, Wk2, Wv, Wproj, lq1, lk1, lq2, lk2, norm_w):
    x = np.asarray(x, np.float32)
    Wq1, Wq2 = np.asarray(Wq1, np.float32), np.asarray(Wq2, np.float32)
    Wk1, Wk2 = np.asarray(Wk1, np.float32), np.asarray(Wk2, np.float32)
    Wv, Wproj = np.asarray(Wv, np.float32), np.asarray(Wproj, np.float32)
    lq1, lk1 = np.asarray(lq1, np.float32), np.asarray(lk1, np.float32)
    lq2, lk2 = np.asarray(lq2, np.float32), np.asarray(lk2, np.float32)
    norm_w = np.asarray(norm_w, np.float32)

    lam = float(
        np.exp(np.sum(lq1 * lk1, dtype=np.float64))
        - np.exp(np.sum(lq2 * lk2, dtype=np.float64))
        + LAMBDA_INIT
    )

    # norm_w * (1 - LAMBDA_INIT) folded into Wproj rows
    normw_t = np.tile(norm_w, HPG) * (1.0 - LAMBDA_INIT)  # [DG]

    hmask = np.zeros((128, 2), np.float32)
    hmask[0:64, 0] = 1.0
    hmask[64:128, 1] = 1.0
    hmaskT = np.ascontiguousarray(hmask.T)
    lam_v = np.full((128, 1), lam, np.float32)

    in_maps = []
    for c in range(8):
        b, hg = divmod(c, HG)
        xt = np.ascontiguousarray(x[b].T)  # [C, T]
        wqm = np.empty((C, 2 * DG), np.float32)
        wkm = np.empty((C, 2 * DG), np.float32)
        for jj in range(HPG):
            gh = HPG * hg + jj
            wqm[:, 128 * jj : 128 * jj + 64] = Wq1[:, 64 * gh : 64 * gh + 64]
            wqm[:, 128 * jj + 64 : 128 * jj + 128] = Wq2[:, 64 * gh : 64 * gh + 64]
            wkm[:, 128 * jj : 128 * jj + 64] = Wk1[:, 64 * gh : 64 * gh + 64]
            wkm[:, 128 * jj + 64 : 128 * jj + 128] = Wk2[:, 64 * gh : 64 * gh + 64]
        wv = np.ascontiguousarray(Wv[:, DG * hg : DG * hg + DG])
        wpj = np.ascontiguousarray(
            normw_t[:, None] * Wproj[DG * hg : DG * hg + DG, :]
        )
        in_maps.append(
            {
                "xt": xt,
                "wqm": wqm,
                "wkm": wkm,
                "wv": wv,
                "wpj": wpj,
                "lam": lam_v,
                "hmask": hmask,
                "hmaskT": hmaskT,
            }
        )

    nc = _get_program()
    res = run_bass_kernel_spmd(nc, in_maps, list(range(8))).results

    diff_attn = np.empty((B, H, T, T), np.float32)
    out = np.zeros((B, T, C), np.float32)
    for c in range(8):
        b, hg = divmod(c, HG)
        diff_attn[b, HPG * hg : HPG * hg + HPG] = res[c]["diff"]
        out[b] += res[c]["outp"]
    return out, diff_attn
